# revision 29
# baseline (speedup 1.0000x reference)
"""LIF spiking-neuron recurrence on Trainium2, 8-core data-parallel SPMD.

Reference recurrence (per neuron, T timesteps):
    h_t = v_{t-1} + (x_t - v_{t-1}) / 2        # TAU = 2.0
    s_t = (h_t >= 1.0)                          # spike
    v_t = (1 - s_t) * h_t                       # hard reset to 0

Kernel computes the algebraically identical scaled form with state
p_t = 2*h_t (bit-exact: *0.5 is exact, the charge add rounds once, the
compare/select are exact):

    p_{t+1} = (p_t < 2) * p_t * 0.5 + x_{t+1}
    s_t     = (p_t >= 2)  as u8

The state update is ONE custom DVE op (LIF_STATE_ANT, registered below
via the concourse custom-DVE extension registry): reset+decay+charge
fused. The fire is split column-wise: 512 columns go through a second
custom op (LIF_PACK2_ANT) that packs two timesteps' spikes into one u8
(halving those columns' store bytes), the rest fire as plain u8 on the
otherwise-idle Pool engine. HBM traffic: 32 MiB in + 6 MiB out per
core, and the store stream is deferred behind the input stream so the
DMA engines run gapless.

Sharding: flatten [B, N] -> 1,048,576 independent neurons, contiguous
1/8 slice per core. Time recurrence stays local per core.
"""

import numpy as np

import concourse.bacc as bacc
import concourse.bass as bass
import concourse.dve_ops as dve_ops
import concourse.mybir as mybir
from concourse.bass_utils import run_bass_kernel_spmd
from concourse.dve_spec import C0, C1, Spec, Src0, Src1, _has_src1
from concourse.dve_spec import lower as dve_lower
from concourse.dve_uop import DveOpSpec
from concourse.tile import TileContext

T = 64
B = 16
N = 65536
P = 128               # SBUF partitions
N_CORES = 8
NEUR = B * N                      # 1048576 neurons
NEUR_PER_CORE = NEUR // N_CORES   # 131072
FD = NEUR_PER_CORE // P           # 1024 fp32 per partition per timestep

# Timesteps batched per DMA transfer (fewer descriptors / less HWDGE+SEQ
# load; transfer bytes unchanged).
NB = 2
X_BUFS = 4   # in-flight input tiles (each NB steps wide)
S_BUFS = 4   # in-flight spike tiles (each NB steps wide)


def _lif_ref(in0, in1, s0, s1, imm2):
    out = (in0 < s0).astype(np.float32) * in0 * np.float32(s1)
    return (out + in1).astype(np.float32)


def register_lif_op():
    """Register the fused LIF state-update op in the concourse custom-DVE
    registry (idempotent): out = (in0 < s0) * in0 * s1 + in1."""
    name = "LIF_STATE_ANT"
    for o in dve_ops.OPS:
        if o.name == name:
            return o
    spec = Spec(
        body=(Src0 < C0) * Src0 * C1 + Src1,
        reference=_lif_ref,
    )
    row = max(dve_ops._SUB_OPCODE_FOR_NAME.values()) + 1
    assert row < 0x20, "custom-DVE opcode rows exhausted"
    dve_ops._SUB_OPCODE_FOR_NAME[name] = row
    shas = {}
    for ver in ("v3", "v4"):
        uops = dve_lower(spec, ver=ver)
        shas[ver] = DveOpSpec(
            name=name, opcode=row, uops=uops, rd1_en=_has_src1(spec)
        ).sha(ver)
    op = dve_ops.DveOp(name, spec, subdim=False, uops_sha=shas)
    dve_ops.OPS.append(op)
    dve_ops.CUSTOM_DVE_SPECS[name] = spec
    return op


def _pack2_ref(in0, in1, s0, s1, imm2):
    return (
        (in0 >= s0).astype(np.float32) + (in1 >= s0).astype(np.float32) * np.float32(s1)
    ).astype(np.float32)


def register_pack_op():
    """Fused 2-step fire+pack: out = (in0 >= s0) + (in1 >= s0)*s1.
    With s1=2 and u8 out this packs two timesteps' spikes into one byte,
    halving spike-store HBM traffic for the packed columns."""
    name = "LIF_PACK2_ANT"
    for o in dve_ops.OPS:
        if o.name == name:
            return o
    spec = Spec(
        body=(Src0 >= C0) + (Src1 >= C0) * C1,
        reference=_pack2_ref,
    )
    row = max(dve_ops._SUB_OPCODE_FOR_NAME.values()) + 1
    assert row < 0x20, "custom-DVE opcode rows exhausted"
    dve_ops._SUB_OPCODE_FOR_NAME[name] = row
    shas = {}
    for ver in ("v3", "v4"):
        uops = dve_lower(spec, ver=ver)
        shas[ver] = DveOpSpec(
            name=name, opcode=row, uops=uops, rd1_en=_has_src1(spec)
        ).sha(ver)
    op = dve_ops.DveOp(name, spec, subdim=False, uops_sha=shas)
    dve_ops.OPS.append(op)
    dve_ops.CUSTOM_DVE_SPECS[name] = spec
    return op


# column split for v11: first PKC columns are bit-packed (2 steps/byte,
# DVE pack op), the remaining PLC fire unpacked on Pool. 480/544 balances
# DVE compute-end (which gates the last stores) against DMA bytes.
PKC = 480
PLC = FD - PKC


def build_lif_bass_v11(
    t_steps: int = T,
    fd: int = FD,
    nb: int = NB,
    x_bufs: int = 8,
    pkc: int = PKC,
) -> bass.Bass:
    """v9 + packed spike output: state update unchanged (fused custom op);
    fire is split into a DVE pack2 op over `pkc` columns (2 steps -> one
    u8, halving those columns' store bytes) and a Pool is_ge over the
    rest. Outputs: s_pk [P, T/2*pkc] u8 (p-major, packed), s_pl
    [P, T*plc] u8 (p-major, plain)."""
    assert t_steps % (2 * nb) == 0
    plc = fd - pkc
    f32 = mybir.dt.float32
    u8 = mybir.dt.uint8
    A = mybir.AluOpType
    lif_op = register_lif_op()
    pack_op = register_pack_op()

    nc = bacc.Bacc(trn_type="TRN2")
    x = nc.dram_tensor("x", [t_steps, P * fd], f32, kind="ExternalInput")
    s_pk = nc.dram_tensor("s_pk", [P, (t_steps // 2) * pkc], u8,
                          kind="ExternalOutput")
    s_pl = nc.dram_tensor("s_pl", [P, t_steps * plc], u8,
                          kind="ExternalOutput")
    xb = x.rearrange("(tb ti) (p f) -> tb p ti f", ti=nb, p=P)
    pkv = s_pk.rearrange("p (tb c) -> p tb c", c=pkc)
    plv = s_pl.rearrange("p (t c) -> p t c", c=plc)

    with TileContext(nc) as tc:
        with (
            tc.tile_pool(name="state", bufs=1) as state,
            tc.tile_pool(name="xin", bufs=x_bufs) as xpool,
        ):
            # 3-deep state rotation: the buffer a state op overwrites was
            # last read two full steps ago, so WAR waits (vs Pool's fire
            # and the pack op) are long satisfied by the time they're checked
            pbufs = [state.tile([P, fd], f32, name=f"p_{i}") for i in range(3)]
            spk_all = state.tile([P, t_steps // 2, pkc], u8, name="spk_all")
            spl_all = state.tile([P, t_steps, plc], u8, name="spl_all")
            dmy = state.tile([P, 1], f32, name="dmy")
            cur = pbufs[0]

            xt_b = None
            x_tiles = []
            for t in range(t_steps):
                tb, ti = divmod(t, nb)
                if ti == 0:
                    xt_b = xpool.tile([P, nb, fd], f32, tag="x", name=f"x_{tb}")
                    if tb == 0:
                        for k in range(nb):
                            nc.sync.dma_start(
                                out=xt_b[:, k : k + 1, :],
                                in_=xb[0, :, k : k + 1, :],
                            )
                    else:
                        nc.sync.dma_start(out=xt_b, in_=xb[tb])
                    x_tiles.append(xt_b)
                if t == 0:
                    # v_{-1} = 0, so p_0 = x_0 exactly: no state op needed —
                    # step 0 reads the x tile directly, shortening the
                    # serial DVE chain (which gates the program end)
                    cur = x_tiles[0][:, 0, :]
                    nc.gpsimd.tensor_scalar(
                        spl_all[:, 0, :], cur[:, pkc:], 2.0, None, A.is_ge
                    )
                    continue
                nxt = pbufs[t % 3]
                # state: nxt = (cur < 2)*cur*0.5 + x_t   (fused reset+charge)
                nc.vector._custom_dve(
                    lif_op, out=nxt, in0=cur, in1=xt_b[:, ti, :], s0=2.0, s1=0.5
                )
                # fire, plain columns on Pool
                nc.gpsimd.tensor_scalar(
                    spl_all[:, t, :], nxt[:, pkc:], 2.0, None, A.is_ge
                )
                if t % 2 == 1:
                    # fire+pack both steps' packed columns: cur still holds
                    # step t-1's state (x_0 itself for the first pack),
                    # nxt holds step t's
                    nc.vector._custom_dve(
                        pack_op, out=spk_all[:, t // 2, :],
                        in0=cur[:, :pkc], in1=nxt[:, :pkc], s0=2.0, s1=2.0,
                    )
                cur = nxt

            # gate the store stream behind the final load (ACT FIFO; SP
            # stores are behind the ins on SP's FIFO already)
            nc.scalar.copy(dmy, x_tiles[-1][:, nb - 1, :1])

            # stores: descending chunk sizes — big chunks while fires are
            # plentiful, tiny chunks only at the fire-gated very end —
            # emitted in gate order, alternating SP/ACT issue queues so
            # per-DMA issue overhead overlaps transfers
            pl_chunks = [(0, 16), (16, 16), (32, 16), (48, 8),
                         (56, 7), (63, 1)]
            pk_chunks = [(0, 8), (8, 8), (16, 8), (24, 4),
                         (28, 3), (31, 1)]
            stores = [("pl", o, w, o + w - 1) for o, w in pl_chunks]
            stores += [("pk", o, w, 2 * (o + w) - 1) for o, w in pk_chunks]
            stores.sort(key=lambda r: r[3])
            q = [nc.scalar, nc.sync]
            for j, (kind, o, w, gate) in enumerate(stores):
                eng = q[j % 2]
                if kind == "pl":
                    eng.dma_start(
                        out=plv[:, o : o + w, :], in_=spl_all[:, o : o + w, :]
                    )
                else:
                    eng.dma_start(
                        out=pkv[:, o : o + w, :], in_=spk_all[:, o : o + w, :]
                    )

    nc.finalize()
    return nc


def build_lif_bass_v9(
    t_steps: int = T,
    fd: int = FD,
    nb: int = NB,
    x_bufs: int = X_BUFS,
    s_bufs: int = S_BUFS,
    fire_dve_cols: int = FD,
) -> bass.Bass:
    """Per-core kernel: x [t_steps, P*fd] f32 -> s [t_steps, P*fd] u8.

    Per step: one fused custom-DVE state op + one 2x-mode tensor_scalar
    fire. State ping-pongs between two SBUF tiles so the fire of step t
    and the state op of step t+1 never alias.
    """
    assert t_steps % nb == 0
    f32 = mybir.dt.float32
    u8 = mybir.dt.uint8
    A = mybir.AluOpType
    lif_op = register_lif_op()

    nc = bacc.Bacc(trn_type="TRN2")
    x = nc.dram_tensor("x", [t_steps, P * fd], f32, kind="ExternalInput")
    s = nc.dram_tensor("s", [t_steps, P * fd], u8, kind="ExternalOutput")
    xb = x.rearrange("(tb ti) (p f) -> tb p ti f", ti=nb, p=P)

    # Spikes accumulate in ONE big SBUF tile (64 KiB/partition) and are
    # stored to HBM only after every x load has issued: total DMA traffic
    # (116.5us) exceeds DVE compute (111us), so the schedule end is
    # DMA-bound, and any store that interleaves with the input stream
    # delays x arrivals and stalls compute. A tiny ACT op that reads the
    # last x tile gates the store stream (ACT's queue is FIFO) behind the
    # final load; the tail is stored per-step so the last, fire-gated
    # store is small.
    nbo = 8
    sb = s.rearrange("(tb ti) (p f) -> tb p ti f", ti=nbo, p=P)

    with TileContext(nc) as tc:
        with (
            tc.tile_pool(name="state", bufs=1) as state,
            tc.tile_pool(name="xin", bufs=x_bufs) as xpool,
        ):
            pa = state.tile([P, fd], f32, name="p_a")
            pb = state.tile([P, fd], f32, name="p_b")
            s_all = state.tile([P, t_steps, fd], u8, name="s_all")
            dmy = state.tile([P, 1], f32, name="dmy")
            nc.vector.memset(pa, 0.0)
            cur = pa

            xt_b = None
            x_tiles = []
            for t in range(t_steps):
                tb, ti = divmod(t, nb)
                if ti == 0:
                    xt_b = xpool.tile([P, nb, fd], f32, tag="x", name=f"x_{tb}")
                    if tb == 0:
                        # split the first load per-step so compute starts
                        # after 1/nb of the transfer
                        for k in range(nb):
                            nc.sync.dma_start(
                                out=xt_b[:, k : k + 1, :],
                                in_=xb[0, :, k : k + 1, :],
                            )
                    else:
                        nc.sync.dma_start(out=xt_b, in_=xb[tb])
                    x_tiles.append(xt_b)
                nxt = pb if cur is pa else pa
                # state: nxt = (cur < 2)*cur*0.5 + x_t   (fused reset+charge)
                nc.vector._custom_dve(
                    lif_op, out=nxt, in0=cur, in1=xt_b[:, ti, :], s0=2.0, s1=0.5
                )
                # fire: s_t = (nxt >= 2) as u8 — column-split between DVE
                # and the otherwise-idle Pool engine so DVE (the pacing
                # engine) finishes before the DMA window closes
                fc = fire_dve_cols
                nc.vector.tensor_scalar(
                    s_all[:, t, :fc], nxt[:, :fc], 2.0, None, A.is_ge
                )
                if fc < fd:
                    nc.gpsimd.tensor_scalar(
                        s_all[:, t, fc:], nxt[:, fc:], 2.0, None, A.is_ge
                    )
                cur = nxt

            # gate: ACT reads the last x tile, so the stores queued behind
            # this on ACT's FIFO cannot start before the final load landed
            # (stores on SP's queue are gated for free: FIFO behind the ins)
            nc.scalar.copy(dmy, x_tiles[-1][:, nb - 1, :1])

            tail = 8
            for j, o in enumerate(range(0, t_steps - tail, nbo)):
                eng = nc.scalar if j % 2 == 0 else nc.sync
                eng.dma_start(out=sb[o // nbo], in_=s_all[:, o : o + nbo, :])
            for j, t in enumerate(range(t_steps - tail, t_steps)):
                tb, ti = divmod(t, nbo)
                # alternate issue queues so the ~720ns per-DMA issue path
                # overlaps across the small tail stores
                eng = nc.scalar if j % 2 == 0 else nc.sync
                eng.dma_start(
                    out=sb[tb, :, ti : ti + 1, :],
                    in_=s_all[:, t : t + 1, :],
                )

    nc.finalize()
    return nc


_NC_CACHE: dict = {}

DESIGN = "v11"   # "v9" (plain u8 out) | "v11" (2-step packed out)


def _get_nc():
    if DESIGN not in _NC_CACHE:
        if DESIGN == "v11":
            _NC_CACHE[DESIGN] = build_lif_bass_v11()
        else:
            _NC_CACHE[DESIGN] = build_lif_bass_v9(x_bufs=8, fire_dve_cols=512)
    return _NC_CACHE[DESIGN]


def kernel(x: np.ndarray) -> np.ndarray:
    assert x.shape == (T, B, N), x.shape
    x = np.ascontiguousarray(x, dtype=np.float32)
    xf = x.reshape(T, NEUR)

    in_maps = []
    for c in range(N_CORES):
        lo = c * NEUR_PER_CORE
        shard = np.ascontiguousarray(xf[:, lo : lo + NEUR_PER_CORE])
        in_maps.append({"x": shard})

    nc = _get_nc()
    res = run_bass_kernel_spmd(nc, in_maps, core_ids=list(range(N_CORES)))

    out = np.empty((T, NEUR), dtype=np.float32)
    for c in range(N_CORES):
        lo = c * NEUR_PER_CORE
        r = res.results[c]
        if DESIGN == "v11":
            # reassemble: packed columns carry 2 steps/byte (bit0 = even
            # step, bit1 = odd step), plain columns are u8 {0,1}
            sc = np.empty((T, P, FD), dtype=np.float32)
            pk = r["s_pk"].reshape(P, T // 2, PKC)
            sc[0::2, :, :PKC] = (pk & 1).transpose(1, 0, 2)
            sc[1::2, :, :PKC] = (pk >> 1).transpose(1, 0, 2)
            pl = r["s_pl"].reshape(P, T, PLC)
            sc[:, :, PKC:] = pl.transpose(1, 0, 2)
            out[:, lo : lo + NEUR_PER_CORE] = sc.reshape(T, NEUR_PER_CORE)
        else:
            out[:, lo : lo + NEUR_PER_CORE] = r["s"].astype(np.float32)
    return out.reshape(T, B, N)
